# revision 1
# baseline (speedup 1.0000x reference)
"""AdditiveAttention (d2l-style) distributed Bass kernel for 8 TRN2 NeuronCores.

Full inputs in, full output out.

Sharding (balanced, valid-length aware): batches are sorted by their k-tile
count nk_b = ceil(valid_len/128) descending; every core takes query rows
[32c, 32c+32) of EVERY batch. All 8 cores then carry identical-shape work
(SPMD), and key tiles beyond each batch's valid length are skipped entirely —
the in-tile remainder is handled by a 0/1 mask column. The graph is built per
sorted nk tuple (cached); (4,4,4,4) is the dense case.

Per-core pipeline (all fp32 accumulation, bf16 data path):
  qT[h,lq] = (q @ Wq).T, kT_b[h,lk] = (k_b @ Wk).T     (PE, bf16, d-tiled)
  feat[h, (lq,lk)] = tanh(kT_b[h,:] + qT[h,lq])        (DVE adds, q col as
                                                        per-partition scalar;
                                                        ACT tanh in place on
                                                        big chunks)
  scoresT_t[lk, lq] = wv . feat[:, lq, lk-tile t]      (PE: feat block
                                                        stationary, wv moving,
                                                        N=1 -> one scoresT
                                                        column per matmul)
  pT_t = exp(scoresT_t)     (no max subtraction needed: |scores| <= sum|wv|,
                             exp can't overflow; softmax shift cancels)
  out = (pT.T @ (v ⊙ m)) / (pT.T @ m)                  (per-batch 32-row
                                                        accumulated matmuls at
                                                        col_grp 32*bi)

Masking is exactly equivalent to the reference's -1e6 additive mask: excluded
key positions contribute 0 to both numerator and denominator.
"""

import math
import sys

sys.path.insert(0, "/opt/trn_rl_repo")

from contextlib import ExitStack

import numpy as np

import concourse.bass as bass
import concourse.mybir as mybir
from concourse import bass_utils, tile

B, LQ, LK, DQ, DK, DV, H = 4, 256, 512, 256, 256, 256, 128
NCORES = 8
RPB = 32  # query rows per (core, batch)
F32 = mybir.dt.float32
BF16 = mybir.dt.bfloat16
AF = mybir.ActivationFunctionType


def _blob_layout(nks):
    """Column offsets of the three packed all-bf16 input blobs for a given
    sorted nk tuple. Each blob is one DMA on its own queue/semaphore with a
    disjoint consumer set (walrus accepts only one sync-wait per instruction,
    so no instruction may depend on two queues). The two main blobs hold the
    d-tile halves of the projection inputs — each projection matmul touches
    exactly one — so the d0 matmuls can start while d1 is still in flight.
    The vals blob is only needed for the output tail."""
    nktot = sum(nks)
    off = {}
    o = 0
    off["qT"] = o  # per-d-tile: [128 partitions, 128 lq cols]
    o += 128
    off["kT"] = o  # per-d-tile, per-batch segments
    o += nktot * 128
    off["Wq"] = o
    o += H
    off["Wk"] = o
    o += H
    off["wv"] = o  # only read from blob A; dead padding in blob B
    o += 1
    main_cols = o
    o = 0
    off["v"] = o
    o += nktot * DV  # per (batch, tile)
    off["m"] = o
    o += nktot  # per (batch, tile)
    return off, main_cols, o


def _row_chunks(bi, nbatch):
    """Row-chunk plan for batch bi: small leading chunks so the first tanh
    starts early; small trailing chunks so the exp/output tail starts early."""
    if bi == 0:
        # smooth growth: tanh(chunk c) must outlast the adds of chunk c+1 or
        # ACT stalls at the size jump (measured ~0.7us at a 8->16 jump)
        return [4, 4, 8, 8, 8]
    if bi == nbatch - 1:
        return [16, 8, 4, 4]
    return [16, 16]


def _body(ctx: ExitStack, tc: "tile.TileContext", aps: dict, nks):
    nc = tc.nc
    nktot = sum(nks)
    ntiles = nks[0]
    segs = [sum(nks[:i]) for i in range(len(nks))]  # k-tile offset per batch
    off, main_cols, vals_cols = _blob_layout(nks)

    const = ctx.enter_context(tc.tile_pool(name="const", bufs=1))
    work = ctx.enter_context(tc.tile_pool(name="work", bufs=1))
    proj_ps = ctx.enter_context(tc.tile_pool(name="proj_ps", bufs=2, space="PSUM"))
    acc_ps = ctx.enter_context(tc.tile_pool(name="acc_ps", bufs=1, space="PSUM"))

    blob_a = const.tile([128, main_cols], BF16, tag="blob_a")
    nc.sync.dma_start(blob_a[:], aps["blob_a"][:, :])
    blob_b = const.tile([128, main_cols], BF16, tag="blob_b")
    nc.sync.dma_start(blob_b[:], aps["blob_b"][:, :])
    vblob = const.tile([128, vals_cols], BF16, tag="vblob")
    nc.sync.dma_start(vblob[:], aps["vblob"][:, :])

    mains = [blob_a, blob_b]

    def slab(i, o, w):
        return mains[i][:, o : o + w]

    def vslab(o, w):
        return vblob[:, o : o + w]

    qT_in = [slab(i, off["qT"], 128) for i in range(2)]
    kT_in = [slab(i, off["kT"], nktot * 128) for i in range(2)]
    Wq_sb = [slab(i, off["Wq"], H) for i in range(2)]
    Wk_sb = [slab(i, off["Wk"], H) for i in range(2)]
    wv_bf = slab(0, off["wv"], 1)

    # Projections first (they gate the whole pipeline). The PSUM->SBUF
    # readbacks that gate the FIRST adds (qTf, batch-0 kTb) go on the Scalar
    # engine, which is otherwise idle until its first tanh — keeping the DVE
    # queue free to start the adds immediately. Later batches' kTb casts are
    # emitted just before their own chunks (DVE has slack by then).
    qT_p = proj_ps.tile([H, 128], F32, tag="projk")
    nc.tensor.matmul(qT_p[:], lhsT=Wq_sb[0], rhs=qT_in[0], start=True, stop=False)
    nc.tensor.matmul(qT_p[:], lhsT=Wq_sb[1], rhs=qT_in[1], start=False, stop=True)
    # stays f32: read back as the per-partition scalar operand of the adds
    qTf = const.tile([H, 128], F32, tag="qTf")
    nc.scalar.copy(qTf[:], qT_p[:])

    kTb = const.tile([H, nktot * 128], BF16, tag="kTb")
    kT_ps = []
    for bi, nk in enumerate(nks):
        w = nk * 128
        kT_p = proj_ps.tile([H, 512], F32, tag="projk")
        nc.tensor.matmul(
            kT_p[:, 0:w], lhsT=Wk_sb[0], rhs=kT_in[0][:, segs[bi] * 128 : segs[bi] * 128 + w],
            start=True, stop=False,
        )
        nc.tensor.matmul(
            kT_p[:, 0:w], lhsT=Wk_sb[1], rhs=kT_in[1][:, segs[bi] * 128 : segs[bi] * 128 + w],
            start=False, stop=True,
        )
        kT_ps.append(kT_p)
        if bi == 0:
            nc.scalar.copy(kTb[:, 0:w], kT_p[:, 0:w])

    # Main loop: per sorted batch, per row-chunk: DVE adds (FD = nk*128,
    # q col as per-partition scalar), tanh IN PLACE, then one column-matvec
    # per (row, k-tile). Chunk buffers are never reused, so no WAR/WAW waits.
    scoresT_ps = []
    for t in range(ntiles):
        sc = acc_ps.tile([128, 128], F32, tag=f"scT{t}")
        scoresT_ps.append(sc)

    nchunk = 0
    for bi, nk in enumerate(nks):
        fd = nk * 128
        kslab = kTb[:, segs[bi] * 128 : segs[bi] * 128 + fd]
        if bi > 0:
            nc.vector.tensor_copy(kslab, kT_ps[bi][:, 0:fd])
        r0 = 0
        for gsz in _row_chunks(bi, len(nks)):
            feat = work.tile([H, gsz * fd], BF16, tag=f"ch{nchunk}")
            nchunk += 1
            for g in range(gsz):
                lq = RPB * bi + r0 + g
                nc.vector.tensor_scalar_add(
                    feat[:, fd * g : fd * (g + 1)], kslab, qTf[:, lq : lq + 1]
                )
            tanh_inst = nc.scalar.activation(feat[:], feat[:], AF.Tanh)
            if nchunk == 2:
                first_tanhs = tanh_inst  # gate for low-priority vb fills
            for t in range(nk):
                for g in range(gsz):
                    lq = RPB * bi + r0 + g
                    nc.tensor.matmul(
                        scoresT_ps[t][0:128, lq : lq + 1],
                        lhsT=feat[:, fd * g + 128 * t : fd * g + 128 * (t + 1)],
                        rhs=wv_bf,
                        start=True,
                        stop=True,
                    )
            r0 += gsz

    # masked values: fill DVE idle slots, but only AFTER the ramp — without
    # the explicit dep the scheduler slots these into the DVE exactly when
    # the first adds become ready, delaying tanh0 by ~1us (measured).
    vb, mb = [], []
    for i in range(nktot):
        mcol = vslab(off["m"] + i, 1)
        x = const.tile([128, DV], BF16, tag=f"vb{i}")
        tt = nc.vector.tensor_tensor(
            out=x[:],
            in0=vslab(off["v"] + i * DV, DV),
            in1=mcol.broadcast_to([128, DV]),
            op=mybir.AluOpType.mult,
        )
        tile.add_dep_helper(tt.ins, first_tanhs.ins, reason="defer vb fills past the ramp")
        vb.append(x)
        mb.append(mcol)

    # pT_t = exp(scoresT_t) on the valid column prefix (batches are sorted by
    # nk desc, so tiles beyond a batch's nk form an untouched suffix).
    pT_sb = []
    for t in range(ntiles):
        valid = RPB * sum(1 for x in nks if x > t)
        s = work.tile([128, 128], BF16, tag=f"pT{t}")
        nc.scalar.activation(s[:, 0:valid], scoresT_ps[t][:, 0:valid], AF.Exp)
        pT_sb.append(s)

    # out[32bi:32bi+32, :] = sum_t pT_t[:, block].T @ vb ; denominator via m
    out_ps = acc_ps.tile([128, DV], F32, tag="out_ps")
    sum_ps = acc_ps.tile([128, 1], F32, tag="sum_ps")
    for bi, nk in enumerate(nks):
        sl = slice(RPB * bi, RPB * bi + RPB)
        for t in range(nk):
            nc.tensor.matmul(
                out_ps[sl, :],
                lhsT=pT_sb[t][:, sl],
                rhs=vb[segs[bi] + t][:],
                start=(t == 0),
                stop=(t == nk - 1),
                tile_position=(0, RPB * bi),
            )
        for t in range(nk):
            nc.tensor.matmul(
                sum_ps[sl, 0:1],
                lhsT=pT_sb[t][:, sl],
                rhs=mb[segs[bi] + t],
                start=(t == 0),
                stop=(t == nk - 1),
                tile_position=(0, RPB * bi),
            )

    rs = const.tile([128, 1], F32, tag="rs")
    nc.vector.reciprocal(rs[:], sum_ps[:])
    out_sb = const.tile([128, DV], F32, tag="out_sb")
    nc.vector.tensor_scalar_mul(out_sb[:], out_ps[:], rs[:, 0:1])
    nc.sync.dma_start(aps["out"][:, :], out_sb[:])


def build_graph(nks) -> bass.Bass:
    nc = bass.Bass("TRN2", target_bir_lowering=False, debug=False)
    _, main_cols, vals_cols = _blob_layout(nks)
    aps = {
        "blob_a": nc.dram_tensor("blob_a", [128, main_cols], BF16, kind="ExternalInput").ap(),
        "blob_b": nc.dram_tensor("blob_b", [128, main_cols], BF16, kind="ExternalInput").ap(),
        "vblob": nc.dram_tensor("vblob", [128, vals_cols], BF16, kind="ExternalInput").ap(),
        "out": nc.dram_tensor("out", [128, DV], F32, kind="ExternalOutput").ap(),
    }
    with tile.TileContext(nc) as tc:
        with ExitStack() as ctx:
            _body(ctx, tc, aps, nks)
    _split_multi_waits(nc)
    return nc


def _split_multi_waits(nc):
    """This walrus build accepts only ONE sync-wait per instruction (every
    TPB struct's setupSyncWait rejects more). Tile emits instructions with
    several waits. Legalize: keep one wait on the instruction and hoist the
    rest onto freshly inserted same-engine NOPs placed immediately before it
    in the basic block — identical blocking semantics, no reordering."""
    n = 0
    for bb in nc.m.functions[0].blocks:
        insts = bb.instructions
        out = []
        for inst in insts:
            si = inst.sync_info
            if si is not None and si.on_wait and len(si.on_wait) > 1:
                waits = list(si.on_wait)
                for w in waits[:-1]:
                    nop = mybir.InstNoOp(
                        name=f"{inst.name}-wsplit{n}",
                        text_hint="waitsplit",
                        bass_nofuse=True,
                        engine=inst.engine,
                        sync_info=mybir.SyncInfo(on_wait=[w], on_update=[]),
                    )
                    nc.register_instruction(nop)
                    out.append(nop)
                    n += 1
                inst.sync_info = mybir.SyncInfo(
                    on_wait=[waits[-1]], on_update=si.on_update
                )
            out.append(inst)
        if n:
            bb.instructions = out


# nk tuples whose NEFFs were pre-compiled into the on-disk neuron cache.
# Unlisted tuples fall back to the dense (4,4,4,4) graph — always correct
# (the 0/1 mask column handles everything), just without the tile skipping —
# so a cold harness never pays an unexpected multi-minute walrus compile.
PRECOMPILED_NKS = {
    (4, 4, 4, 4),
    (4, 4, 4, 3),
    (4, 4, 4, 2),
    (4, 4, 4, 1),
    (4, 4, 3, 3),
    (4, 4, 3, 2),
    (4, 4, 3, 1),
    (4, 4, 2, 2),
    (4, 4, 2, 1),
    (4, 4, 1, 1),
    (4, 3, 3, 3),
    (4, 3, 3, 2),
    (4, 3, 3, 1),
    (4, 3, 2, 2),
    (4, 3, 2, 1),
    (4, 3, 1, 1),
    (4, 2, 2, 2),
    (4, 2, 2, 1),
    (4, 2, 1, 1),
    (4, 1, 1, 1),
    (3, 3, 3, 3),
    (3, 3, 3, 2),
    (3, 3, 3, 1),
    (3, 3, 2, 2),
    (3, 3, 2, 1),
    (3, 3, 1, 1),
    (3, 2, 2, 2),
    (3, 2, 2, 1),
    (3, 2, 1, 1),
    (3, 1, 1, 1),
    (2, 2, 2, 2),
    (2, 2, 2, 1),
    (2, 2, 1, 1),
    (2, 1, 1, 1),
    (1, 1, 1, 1),
}


def _plan(valid_lens):
    nk = [min(4, max(1, math.ceil(int(v) / 128))) for v in valid_lens]
    order = sorted(range(B), key=lambda b: -nk[b])
    nks = tuple(nk[b] for b in order)
    if PRECOMPILED_NKS and nks not in PRECOMPILED_NKS:
        nks = (4, 4, 4, 4)
    return order, nks


def make_in_maps(queries, keys, values, Wq, Wk, wv, valid_lens, order, nks):
    import ml_dtypes

    bf = ml_dtypes.bfloat16
    f = np.float32
    queries = np.asarray(queries, f)
    keys = np.asarray(keys, f)
    values = np.asarray(values, f)
    Wqf = np.asarray(Wq, f)
    Wkf = np.asarray(Wk, f)
    wvf = np.asarray(wv, f).reshape(H)
    off, main_cols, vals_cols = _blob_layout(nks)
    segs = [sum(nks[:i]) for i in range(len(nks))]

    # two main blobs: d-tile halves of the projection inputs
    bases = [np.zeros((128, main_cols), f) for _ in range(2)]
    for i in range(2):
        bases[i][:, off["Wq"] : off["Wq"] + H] = Wqf[128 * i : 128 * (i + 1)]
        bases[i][:, off["Wk"] : off["Wk"] + H] = Wkf[128 * i : 128 * (i + 1)]
    bases[0][:, off["wv"]] = wvf
    vbase = np.empty((128, vals_cols), f)
    for bi, b in enumerate(order):
        nk = nks[bi]
        kT = keys[b].T  # [DK, LK]
        m = (np.arange(LK) < int(valid_lens[b])).astype(f)
        s = segs[bi] * 128
        for i in range(2):
            bases[i][:, off["kT"] + s : off["kT"] + s + nk * 128] = kT[
                128 * i : 128 * (i + 1), : nk * 128
            ]
        for t in range(nk):
            j = segs[bi] + t
            vbase[:, off["v"] + j * DV : off["v"] + (j + 1) * DV] = values[
                b, 128 * t : 128 * (t + 1), :
            ]
            vbase[:, off["m"] + j] = m[128 * t : 128 * (t + 1)]

    in_maps = []
    for c in range(NCORES):
        ab = [bases[0].copy(), bases[1].copy()]
        for bi, b in enumerate(order):
            qT = queries[b, RPB * c : RPB * (c + 1), :].T  # [DQ, 32]
            for i in range(2):
                ab[i][:, off["qT"] + RPB * bi : off["qT"] + RPB * (bi + 1)] = qT[
                    128 * i : 128 * (i + 1)
                ]
        in_maps.append(
            {"blob_a": ab[0].astype(bf), "blob_b": ab[1].astype(bf), "vblob": vbase.astype(bf)}
        )
    return in_maps


_CACHE: dict = {}


def kernel(queries, keys, values, Wq, Wk, wv, valid_lens, _trace=False, _trace_kwargs=None):
    order, nks = _plan(valid_lens)
    if nks not in _CACHE:
        _CACHE[nks] = build_graph(nks)
    nc = _CACHE[nks]
    in_maps = make_in_maps(queries, keys, values, Wq, Wk, wv, valid_lens, order, nks)
    res = bass_utils.run_bass_kernel_spmd(
        nc,
        in_maps,
        core_ids=list(range(NCORES)),
        trace=_trace,
        **(_trace_kwargs or {}),
    )
    out = np.empty((B, LQ, DV), dtype=np.float32)
    for c in range(NCORES):
        o = res.results[c]["out"]
        for bi, b in enumerate(order):
            out[b, RPB * c : RPB * (c + 1), :] = o[RPB * bi : RPB * (bi + 1), :]
    if _trace:
        return out, res
    return out

